# revision 1
# baseline (speedup 1.0000x reference)
"""MoE gate routing kernel for Trainium2 (8 NeuronCores, data-parallel over tokens).

Computes, for x[8192,7168], weight[256,7168], bias[256]:
    scores = sigmoid(x @ weight.T + bias)            # [N, 256]
    group top-2 sums over 8 groups of 32 -> pick best group
    top-8 experts within best group (global indices), weights = renormalized
    sigmoid scores * 2.5
Returns (w [8192,8] f32, idx [8192,8] i32).

Strategy: shard tokens 8-way (1024/core). The kernel is HBM-bound: fp32 x is
29.4 MB/core, and cheaper encodings of x flip router top-k decisions past the
2e-2 gate (fp16: 2.27e-2; int16-only: 2.06e-2 -- both hardware-measured, the
idx rel-err metric is dominated by a handful of group-flip tokens). So x ships
as int16 + int8 residual (24-bit fixed point, reconstruction exact to ~1e-6 =
below fp32 accumulation noise) at 3 B/elem = 22 MB, and weight ships as raw
fp32 typed f32r (7.3 MB). Accuracy is then identical to the fp32r baseline
(1.35e-2) while DMA drops 20%.

On device the reconstruction x = i16*s + i8*(s/256) runs as two passes over
otherwise-idle engines: pass 1 on ScalarE (activation copy-with-scale), pass 2
on Pool (scalar_tensor_tensor fused multiply-add) writing f32r for the
full-rate fp32r matmul. VectorE keeps the top-k chain. Work is quartered so
reconstruction pipelines against the DMA stream.

x is host-packed as [NBUF, 128, KC, 128] so each half-buffer DMA is one
contiguous descriptor per partition (full DMA rate).
"""

import sys

sys.path.insert(0, "/opt/trn_rl_repo")

from concurrent.futures import ThreadPoolExecutor

import numpy as np

import concourse.bass as bass
from concourse import bacc
import concourse.mybir as mybir
from concourse.bass_utils import run_bass_kernel_spmd
from concourse.tile import TileContext

N_CORES = 8
N_TOK = 8192
TOK_PC = N_TOK // N_CORES  # 1024 tokens per core
D = 7168
E = 256
G = 8  # groups
EPG = E // G  # 32 experts per group
TOPK = 8
ROUTE_SCALE = 2.5
KC = D // 128  # 56 k-chunks
KCH = KC // 2  # 28 k-chunks per half-buffer
KQ = KC // 4  # 14 k-chunks per recon quarter
XBUF_T = 128  # tokens per x buffer / subtile
NBUF = TOK_PC // XBUF_T  # 8 buffers/subtiles per core

f32 = mybir.dt.float32
f32r = mybir.dt.float32r
i16 = mybir.dt.int16
i8 = mybir.dt.int8
i32 = mybir.dt.int32
u32 = mybir.dt.uint32
AX = mybir.AxisListType
OP = mybir.AluOpType
ACTF = mybir.ActivationFunctionType

_cache = {}
LAST_RESULT = None  # BassKernelResults of the most recent run (for test harness)


def _build():
    nc = bacc.Bacc(None, target_bir_lowering=False)

    # x packed [NBUF, 128, KC, XBUF_T] flattened to 2D, hi/lo streams
    xh = nc.declare_dram_parameter("xh", [NBUF * 128, KC * XBUF_T], i16, isOutput=False)
    xl = nc.declare_dram_parameter("xl", [NBUF * 128, KC * XBUF_T], i8, isOutput=False)
    wT = nc.declare_dram_parameter("wT", [D, E], f32, isOutput=False)
    bias = nc.declare_dram_parameter("bias", [1, E], f32, isOutput=False)
    w_out = nc.declare_dram_parameter("w_out", [TOK_PC, TOPK], f32, isOutput=True)
    idx_out = nc.declare_dram_parameter("idx_out", [TOK_PC, TOPK], i32, isOutput=True)

    xh_v = xh.rearrange("(b p) (c n) -> b p c n", p=128, c=KC)
    xl_v = xl.rearrange("(b p) (c n) -> b p c n", p=128, c=KC)
    wT_v = wT.rearrange("(c p) e -> p c e", p=128)  # [128, KC, E]

    with TileContext(nc) as tc:
        with (
            tc.tile_pool(name="const", bufs=1) as cpool,
            tc.tile_pool(name="xh", bufs=4) as hpool,
            tc.tile_pool(name="xl", bufs=4) as lpool,
            tc.tile_pool(name="xt", bufs=4) as tpool,
            tc.tile_pool(name="x32", bufs=6) as xpool,
            tc.tile_pool(name="sb", bufs=2) as spool,
            tc.tile_pool(name="small", bufs=3) as mpool,
            tc.tile_pool(name="out", bufs=3) as opool,
            tc.tile_pool(name="psum", bufs=6, space="PSUM") as ppool,
        ):
            def dma_xpiece(s, h):
                ht = hpool.tile([128, KCH, XBUF_T], i16, tag="xh")
                nc.sync.dma_start(out=ht, in_=xh_v[s, :, h * KCH : (h + 1) * KCH, :])
                lt = lpool.tile([128, KCH, XBUF_T], i8, tag="xl")
                nc.sync.dma_start(out=lt, in_=xl_v[s, :, h * KCH : (h + 1) * KCH, :])
                return ht, lt

            # x and weight-quarter DMAs interleave so neither stream starves:
            # x0, w0, x1, w1, w2, x2, w3, then x3..x7
            w32 = cpool.tile([128, KC, E], f32r)

            def dma_wq(q):
                sl = slice(q * KQ, (q + 1) * KQ)
                nc.sync.dma_start(out=w32[:, sl, :], in_=wT_v[:, sl, :].bitcast(f32r))

            pre = {0: [dma_xpiece(0, 0), dma_xpiece(0, 1)]}
            bias_sb = cpool.tile([1, E], f32)
            nc.sync.dma_start(out=bias_sb, in_=bias[:, :])
            ones_sb = cpool.tile([1, 128], f32)
            nc.vector.memset(ones_sb, 1.0)

            dma_wq(0)
            pre[1] = [dma_xpiece(1, 0), dma_xpiece(1, 1)]
            dma_wq(1)
            pre[2] = [dma_xpiece(2, 0), dma_xpiece(2, 1)]
            dma_wq(2)
            dma_wq(3)

            # accumulate outputs in SBUF; a single DMA pair at the very end
            # keeps the SP sequencer's DMA stream free of data-dependent
            # waits (a per-subtile output DMA would head-of-line-block the
            # later x-input DMAs behind it)
            w8_all = cpool.tile([128, NBUF, TOPK], f32)
            idx_all = cpool.tile([128, NBUF, TOPK], u32)

            def recon_quarter(ht, lt, q, on_pool):
                """x/s = i16 + i8/256 (the x scale is folded into the host-
                scaled weights). ACT casts the hi stream to f32; the lo stream
                fuses in via DVE scalar_tensor_tensor, except one quarter per
                buffer routed to Pool (tensor_scalar+tensor_add pair) to keep
                DVE off the critical cadence. Quarter-sized tiles so matmuls
                start as soon as each quarter is reconstructed."""
                sl = slice(q * KQ, (q + 1) * KQ)
                xt = tpool.tile([128, KQ, XBUF_T], f32, tag="xt")
                nc.scalar.mul(xt, ht[:, sl, :], 1.0)
                x32q = xpool.tile([128, KQ, XBUF_T], f32r, tag="x32")
                if on_pool:
                    lo = tpool.tile([128, KQ, XBUF_T], f32, tag="lo")
                    nc.gpsimd.tensor_scalar(
                        lo, lt[:, sl, :], 0.00390625, None, op0=OP.mult
                    )
                    nc.gpsimd.tensor_add(x32q, lo, xt)
                else:
                    nc.vector.scalar_tensor_tensor(
                        out=x32q,
                        in0=lt[:, sl, :],
                        scalar=0.00390625,
                        in1=xt,
                        op0=OP.mult,
                        op1=OP.add,
                    )
                return x32q

            for s in range(NBUF):
                if s in pre:
                    pieces = pre[s]
                else:
                    pieces = [dma_xpiece(s, 0), dma_xpiece(s, 1)]

                t0 = s * XBUF_T
                ps = ppool.tile([128, E], f32, tag="ps")
                # bias preload: ps[t, e] = 1 * bias[e] (plain f32 matmul)
                nc.tensor.matmul(
                    out=ps, lhsT=ones_sb, rhs=bias_sb, start=True, stop=False
                )
                for h in range(2):
                    ht, lt = pieces[h]
                    for q in range(KCH // KQ):
                        on_pool = (2 * h + q == 3)
                        x32q = recon_quarter(ht, lt, q, on_pool)
                        for c in range(KQ):
                            cc = h * KCH + q * KQ + c
                            nc.tensor.matmul(
                                out=ps,
                                lhsT=x32q[:, c, :],
                                rhs=w32[:, cc, :],
                                start=False,
                                stop=(cc == KC - 1),
                            )

                sig = spool.tile([128, G, EPG], f32, tag="sig")
                nc.scalar.activation(
                    out=sig.rearrange("p g e -> p (g e)"), in_=ps, func=ACTF.Sigmoid
                )
                sig_flat = sig.rearrange("p g e -> p (g e)")

                # group top-2 sum
                m1 = mpool.tile([128, G], f32, tag="m1")
                nc.vector.tensor_reduce(out=m1, in_=sig, axis=AX.X, op=OP.max)
                scr = spool.tile([128, G, EPG], f32, tag="scr")
                nc.vector.match_replace(
                    out=scr.rearrange("p g e -> p (g e)"),
                    in_to_replace=m1,
                    in_values=sig_flat,
                    imm_value=-1e30,
                )
                gs = mpool.tile([128, G], f32, tag="gs")
                nc.vector.tensor_reduce(out=gs, in_=scr, axis=AX.X, op=OP.max)
                nc.vector.tensor_add(gs, gs, m1)  # m1 + m2

                # one-hot of best group -> multiplicative mask
                gmax = mpool.tile([128, 1], f32, tag="gmax")
                nc.vector.tensor_reduce(out=gmax, in_=gs, axis=AX.X, op=OP.max)
                eq = mpool.tile([128, G, 1], f32, tag="eq")
                nc.vector.tensor_scalar(eq[:, :, 0], gs, gmax, None, op0=OP.is_ge)
                # masked scores: kept group unchanged (x1.0), others -> 0.0
                masked = spool.tile([128, G, EPG], f32, tag="masked")
                ba, bb = bass.broadcast_tensor_aps(sig[:, :, :], eq[:, :, :])
                nc.vector.tensor_tensor(out=masked, in0=ba, in1=bb, op=OP.mult)
                masked_flat = masked.rearrange("p g e -> p (g e)")

                vals8 = mpool.tile([128, TOPK], f32, tag="vals8")
                nc.vector.max(out=vals8, in_=masked_flat)
                nc.vector.max_index(
                    out=idx_all[:, s, :], in_max=vals8, in_values=masked_flat
                )

                ssum = mpool.tile([128, 1], f32, tag="ssum")
                nc.vector.tensor_reduce(out=ssum, in_=vals8, axis=AX.X, op=OP.add)
                rcp = mpool.tile([128, 1], f32, tag="rcp")
                nc.vector.reciprocal(out=rcp, in_=ssum)
                nc.vector.tensor_scalar(
                    w8_all[:, s, :], vals8, rcp, ROUTE_SCALE, op0=OP.mult, op1=OP.mult
                )

            w_out_v = w_out.rearrange("(s p) k -> p s k", p=128)
            idx_out_v = idx_out.rearrange("(s p) k -> p s k", p=128)
            nc.sync.dma_start(
                out=idx_out_v[:, : NBUF - 1, :],
                in_=idx_all[:, : NBUF - 1, :].bitcast(i32),
            )
            nc.sync.dma_start(
                out=w_out_v[:, : NBUF - 1, :], in_=w8_all[:, : NBUF - 1, :]
            )
            nc.sync.dma_start(
                out=idx_out_v[:, NBUF - 1 :, :],
                in_=idx_all[:, NBUF - 1 :, :].bitcast(i32),
            )
            nc.sync.dma_start(
                out=w_out_v[:, NBUF - 1 :, :], in_=w8_all[:, NBUF - 1 :, :]
            )
    nc.compile()
    return nc


def kernel(x, weight, bias):
    global LAST_RESULT
    x = np.asarray(x, dtype=np.float32)
    weight = np.asarray(weight, dtype=np.float32)
    bias = np.asarray(bias, dtype=np.float32).reshape(1, E)

    if "nc" not in _cache:
        _cache["nc"] = _build()
    nc = _cache["nc"]

    s_x = float(np.abs(x).max()) / 32767.0
    # x ships as x/s_x (int16 + int8/256); fold s_x into the weights so the
    # device-side reconstruction needs no scale operand
    wTh = np.ascontiguousarray(weight.T * np.float32(s_x))  # [D, E] f32

    def shard(c):
        xs = x[c * TOK_PC : (c + 1) * TOK_PC]  # [1024, D]
        xsc = xs.T / s_x  # [D, 1024]
        hi = np.rint(xsc)
        lo = np.clip(np.rint((xsc - hi) * 256.0), -127, 127).astype(np.int8)
        hi = hi.astype(np.int16)

        def pack(a):
            return np.ascontiguousarray(
                a.reshape(KC, 128, NBUF, XBUF_T)
                .transpose(2, 1, 0, 3)
                .reshape(NBUF * 128, KC * XBUF_T)
            )

        return pack(hi), pack(lo)

    with ThreadPoolExecutor(N_CORES) as ex:
        packed = list(ex.map(shard, range(N_CORES)))

    in_maps = [
        {
            "xh": packed[c][0],
            "xl": packed[c][1],
            "wT": wTh,
            "bias": bias,
        }
        for c in range(N_CORES)
    ]
    res = run_bass_kernel_spmd(nc, in_maps, list(range(N_CORES)))
    LAST_RESULT = res
    w = np.concatenate([res.results[c]["w_out"] for c in range(N_CORES)], axis=0)
    idx = np.concatenate([res.results[c]["idx_out"] for c in range(N_CORES)], axis=0)
    return w, idx.astype(np.int32)



# revision 22
# speedup vs baseline: 1.3520x; 1.3520x over previous
"""MoE gate routing kernel for Trainium2 (8 NeuronCores, data-parallel tokens).

scores = sigmoid(x @ W.T + b); group top-2-sum over 8 groups of 32 picks one
group; top-8 experts inside it; weights = renormalized sigmoid * 2.5.
Returns (w [8192,8] f32, idx [8192,8] i32).

Encoding: the fp32r PE path retains only ~13-14 mantissa bits of its inputs
(any engine write tagged f32r is rounded to that format, and the BIR
verifier requires f32r-tagged producers), so shipping more than ~14 bits of
x is wasted. x therefore ships as a plain int16 payload (2 B/elem) and is
cast on-device to f32r by one tensor_scalar per quarter-buffer (no other
reconstruction work). W ships as int16 + int8 (3 B/elem), reconstructed to
full f32 in a staging tile and rounded once into the f32r W buffer, so its
effective precision equals a raw-f32 DMA at 25% fewer bytes. All scales
fold into the sigmoid scale and a host-precomputed bias row (in payload
units) injected via a ones-column matmul.

Per-core DMA is 2.52 MB = ~57 us at the 360 GB/s DMA ceiling, which is the
bottleneck; PE (fp32r, full 256-wide output rows) needs ~49 us and the
vector engines ~40 us, so both stay off the critical path.
"""

import sys

sys.path.insert(0, "/opt/trn_rl_repo")

from concurrent.futures import ThreadPoolExecutor

import numpy as np

import concourse.bass as bass
from concourse import bacc
import concourse.mybir as mybir
from concourse.bass_utils import run_bass_kernel_spmd
from concourse.tile import TileContext

N_CORES = 8
N_TOK = 8192
TOK_PC = N_TOK // N_CORES  # 1024
D = 7168
E = 256
G = 8
EPG = E // G
TOPK = 8
ROUTE_SCALE = 2.5
KC = D // 128  # 56 k-chunks
KQ = KC // 4  # 14 per quarter
KE = KC // 8  # 7 per eighth (last-buffer tail pieces)
XBUF_T = 128
NBUF = TOK_PC // XBUF_T  # 8

f32 = mybir.dt.float32
f32r = mybir.dt.float32r
i16 = mybir.dt.int16
i8 = mybir.dt.int8
i32 = mybir.dt.int32
u32 = mybir.dt.uint32
AX = mybir.AxisListType
OP = mybir.AluOpType
ACTF = mybir.ActivationFunctionType

_cache = {}
LAST_RESULT = None

# cast engine per (buffer, quarter): phase-A buffers 0-2 pace PE from ACT,
# 3-4 from DVE (behind the w-recon ops there); streaming buffers split
def _cast_eng(s, q):
    if s <= 2:
        return "act"
    return "dve" if q in (0, 2) else "act"
CAST_ENG = [[_cast_eng(s, q) for q in range(4)] for s in range(NBUF)]


def _build(sig_scale):
    nc = bacc.Bacc(None, target_bir_lowering=False)

    xh = nc.declare_dram_parameter("xh", [NBUF * 128, KC * XBUF_T], i16, isOutput=False)
    wh = nc.declare_dram_parameter("wh", [D, E], i16, isOutput=False)
    wl = nc.declare_dram_parameter("wl", [D // 2, 2 * E], i8, isOutput=False)
    bias_p = nc.declare_dram_parameter("bias_p", [1, E], f32, isOutput=False)
    ones_p = nc.declare_dram_parameter("ones_p", [1, 128], f32, isOutput=False)
    w_out = nc.declare_dram_parameter("w_out", [TOK_PC, TOPK], f32, isOutput=True)
    idx_out = nc.declare_dram_parameter("idx_out", [TOK_PC, TOPK], i32, isOutput=True)

    xh_v = xh.rearrange("(b p) (c n) -> b p c n", p=128, c=KC)
    wh_v = wh.rearrange("(c p) e -> p c e", p=128)
    wl_v = wl.rearrange("(c2 p) (two e) -> p c2 two e", p=128, two=2)

    with TileContext(nc) as tc:
        with (
            tc.tile_pool(name="const", bufs=1) as cpool,
            tc.tile_pool(name="xh", bufs=9) as hpool,
            tc.tile_pool(name="x32", bufs=5) as xpool,
            tc.tile_pool(name="wst", bufs=3) as wpool,
            tc.tile_pool(name="wstg", bufs=2) as gpool,
            tc.tile_pool(name="sb", bufs=2) as spool,
            tc.tile_pool(name="small", bufs=3) as mpool,
            tc.tile_pool(name="psum", bufs=8, space="PSUM") as ppool,
        ):
            w32 = cpool.tile([128, KC, E], f32r)
            bias_sb = cpool.tile([1, E], f32r)
            ones_sb = cpool.tile([1, 128], f32r)
            w8_all = cpool.tile([128, NBUF, TOPK], f32)
            idx_all = cpool.tile([128, NBUF, TOPK], u32)

            def dma_x_piece(s, lo, hi):
                """[lo, hi) k-chunk slice of buffer s."""
                ht = hpool.tile([128, KQ, XBUF_T], i16, tag="xh")
                nc.sync.dma_start(out=ht[:, : hi - lo, :], in_=xh_v[s, :, lo:hi, :])
                return ht

            KW = 8  # chunks per w piece (aligned to the wl row-pair packing)

            def dma_w_piece(e):
                sl = slice(e * KW, (e + 1) * KW)
                wht = wpool.tile([128, KW, E], i16, tag="wh")
                nc.sync.dma_start(out=wht, in_=wh_v[:, sl, :])
                wlt = wpool.tile([128, KW, E], i8, tag="wl")
                lv = wl_v[:, e * KW // 2 : (e + 1) * KW // 2, :, :]
                nc.sync.dma_start(
                    out=wlt.rearrange("p (a two) e -> p a two e", two=2), in_=lv
                )
                return wht, wlt

            def recon_w_piece(e, pieces):
                """w32[e] = round_f32r(wh*256 + wl): full-f32 staging, one
                rounding at the final tt write so the int8 residual survives.
                tt alternates DVE/Pool so reconstruction keeps up with the
                wire; the first two pieces go in halves to cut PE's startup
                latency."""
                wht, wlt = pieces
                stg = gpool.tile([128, KW, E], f32, tag="wstg")
                for h in range(2):
                    hs = slice(h * KW // 2, (h + 1) * KW // 2)
                    gsl = slice(e * KW + h * KW // 2, e * KW + (h + 1) * KW // 2)
                    nc.vector.tensor_scalar(
                        stg[:, hs, :], wht[:, hs, :], 256.0, None, op0=OP.mult
                    )
                    eng = nc.vector if h == 0 else nc.gpsimd
                    eng.tensor_tensor(
                        out=w32[:, gsl, :], in0=stg[:, hs, :], in1=wlt[:, hs, :],
                        op=OP.add,
                    )

            def cast_piece(ht, s, Q, nchunk):
                x32q = xpool.tile([128, KQ, XBUF_T], f32r, tag="x32")
                src = ht[:, :nchunk, :]
                dst = x32q[:, :nchunk, :]
                if CAST_ENG[s][Q] == "act":
                    nc.scalar.mul(dst, src, 1.0)
                else:
                    nc.vector.tensor_scalar(dst, src, 1.0, None, op0=OP.mult)
                return x32q

            # ---- prologue ----
            nc.scalar.dma_start(out=ones_sb, in_=ones_p[:, :].bitcast(f32r))
            warm = cpool.tile([1, 16], f32)
            nc.vector.memset(warm, 0.0)
            nc.scalar.activation(out=warm, in_=warm, func=ACTF.Sigmoid)
            nc.scalar.dma_start(out=bias_sb, in_=bias_p[:, :].bitcast(f32r))
            # all 8 buffers sweep in one diagonal wavefront; buffer 7's
            # last quarter is two eighths to shorten the tail chain. w pieces
            # are hand-interleaved into the same queue so the wire delivers
            # each one just before PE first needs it.
            pieces = {}  # (s, lo) -> (tile, lo, hi)
            wv = {}
            for s in range(NBUF):
                for Q in range(4):
                    if s == NBUF - 1 and Q == 3:
                        wv.setdefault(s + Q, []).append((s, 42, 49))
                        wv.setdefault(s + Q, []).append((s, 49, 56))
                    elif s == NBUF - 2 and Q == 3:
                        # buffer 6 finishes a wave early so its topk chain
                        # clears DVE before buffer 7's
                        wv.setdefault(s + Q - 1, []).append((s, Q * KQ, (Q + 1) * KQ))
                    else:
                        wv.setdefault(s + Q, []).append((s, Q * KQ, (Q + 1) * KQ))
            xorder = [sq for w_i in sorted(wv) for sq in wv[w_i]]
            w_after = {0: [1], 1: [2, 3], 2: [4], 3: [5], 4: [6]}
            we = {0: dma_w_piece(0)}
            for i, (s, lo, hi) in enumerate(xorder):
                pieces[(s, lo)] = (dma_x_piece(s, lo, hi), lo, hi)
                for e in w_after.get(i, []):
                    we[e] = dma_w_piece(e)

            # open all PSUM accumulation groups with the bias rank-1 matmuls
            # up front: warms the PE ramp before x arrives and removes
            # mid-stream PE work
            psd = {}
            for s in range(NBUF):
                ps_s = ppool.tile([128, E], f32, tag="ps")
                psd[s] = ps_s
                nc.tensor.matmul(
                    out=ps_s, lhsT=ones_sb, rhs=bias_sb, start=True, stop=False,
                    skip_group_check=True,
                )

            w_reconned = [False] * 7

            def need_w(cc):
                e = cc // KW
                if not w_reconned[e]:
                    w_reconned[e] = True
                    recon_w_piece(e, we[e])

            def mm_piece(s, lo, hi, ht):
                Q = min(lo // KQ, 3)
                x32q = cast_piece(ht, s, Q, hi - lo)
                for c in range(hi - lo):
                    cc = lo + c
                    need_w(cc)
                    nc.tensor.matmul(
                        out=psd[s],
                        lhsT=x32q[:, c, :],
                        rhs=w32[:, cc, :],
                        start=False,
                        stop=(cc == KC - 1),
                        skip_group_check=True,
                    )

            def topk_buf(s):
                sig = spool.tile([128, G, EPG], f32, tag="sig")
                sig_flat = sig.rearrange("p g e -> p (g e)")
                nc.scalar.activation(
                    out=sig_flat, in_=psd.pop(s), func=ACTF.Sigmoid, scale=sig_scale
                )

                m1 = mpool.tile([128, G], f32, tag="m1")
                nc.vector.tensor_reduce(out=m1, in_=sig, axis=AX.X, op=OP.max)
                scr = spool.tile([128, G, EPG], f32, tag="scr")
                nc.vector.match_replace(
                    out=scr.rearrange("p g e -> p (g e)"),
                    in_to_replace=m1,
                    in_values=sig_flat,
                    imm_value=-1e30,
                )
                gs = mpool.tile([128, G], f32, tag="gs")
                nc.vector.tensor_reduce(out=gs, in_=scr, axis=AX.X, op=OP.max)
                nc.vector.tensor_add(gs, gs, m1)

                gmax = mpool.tile([128, 1], f32, tag="gmax")
                nc.vector.tensor_reduce(out=gmax, in_=gs, axis=AX.X, op=OP.max)
                eq = mpool.tile([128, G, 1], f32, tag="eq")
                nc.vector.tensor_scalar(eq[:, :, 0], gs, gmax, None, op0=OP.is_ge)
                masked = spool.tile([128, G, EPG], f32, tag="masked")
                ba, bb = bass.broadcast_tensor_aps(sig[:, :, :], eq[:, :, :])
                nc.vector.tensor_tensor(out=masked, in0=ba, in1=bb, op=OP.mult)
                masked_flat = masked.rearrange("p g e -> p (g e)")

                vals8 = mpool.tile([128, TOPK], f32, tag="vals8")
                nc.vector.max(out=vals8, in_=masked_flat)
                nc.vector.max_index(
                    out=idx_all[:, s, :], in_max=vals8, in_values=masked_flat
                )
                if s == NBUF - 1:
                    iv = idx_out.rearrange("(s p) k -> p s k", p=128)
                    nc.sync.dma_start(
                        out=iv[:, NBUF - 1 :, :],
                        in_=idx_all[:, NBUF - 1 :, :].bitcast(i32),
                    )
                ssum = mpool.tile([128, 1], f32, tag="ssum")
                nc.vector.tensor_reduce(out=ssum, in_=vals8, axis=AX.X, op=OP.add)
                rcp = mpool.tile([128, 1], f32, tag="rcp")
                nc.vector.reciprocal(out=rcp, in_=ssum)
                nc.vector.tensor_scalar(
                    w8_all[:, s, :], vals8, rcp, ROUTE_SCALE, op0=OP.mult, op1=OP.mult
                )

            # ---- wavefront sweep ----
            for w_i in sorted(wv):
                for s, lo, hi in wv[w_i]:
                    ht, lo2, hi2 = pieces.pop((s, lo))
                    mm_piece(s, lo2, hi2, ht)
                if w_i == 8:
                    topk_buf(5)
                    topk_buf(6)
                elif w_i == 9:
                    pass
                elif 3 <= w_i < 3 + NBUF:
                    topk_buf(w_i - 3)
                if w_i - 3 == NBUF - 2:
                    w_out_v = w_out.rearrange("(s p) k -> p s k", p=128)
                    idx_out_v = idx_out.rearrange("(s p) k -> p s k", p=128)
                    nc.sync.dma_start(
                        out=idx_out_v[:, : NBUF - 1, :],
                        in_=idx_all[:, : NBUF - 1, :].bitcast(i32),
                    )
                    nc.sync.dma_start(
                        out=w_out_v[:, : NBUF - 1, :], in_=w8_all[:, : NBUF - 1, :]
                    )

            w_out_v = w_out.rearrange("(s p) k -> p s k", p=128)
            nc.scalar.dma_start(
                out=w_out_v[:, NBUF - 1 :, :], in_=w8_all[:, NBUF - 1 :, :]
            )
    nc.compile()
    return nc


def kernel(x, weight, bias):
    global LAST_RESULT
    x = np.asarray(x, dtype=np.float32)
    weight = np.asarray(weight, dtype=np.float32)
    bias = np.asarray(bias, dtype=np.float32).reshape(1, E)

    s_x = float(np.abs(x).max()) / 32767.0
    wT = weight.T  # [D, E]
    s_w = float(np.abs(wT).max()) / float((1 << 23) - 256)
    Q = np.rint(wT.astype(np.float64) / s_w).astype(np.int64)
    sig_scale = float(np.float32(s_x * s_w))

    Hw = (Q + 128) >> 8
    Lw = (Q - (Hw << 8)).astype(np.int8)
    wh_host = np.ascontiguousarray(Hw.astype(np.int16))
    # wl rows packed in chunk-pairs so each DMA element is 512B
    Lw3 = Lw.reshape(KC, 128, E)
    wl_host = np.ascontiguousarray(
        np.concatenate([Lw3[0::2], Lw3[1::2]], axis=2).reshape(D // 2, 2 * E)
    )
    bias_row = (bias.astype(np.float64) / (s_x * s_w)).astype(np.float32)

    key = sig_scale
    if key not in _cache:
        _cache.clear()
        _cache[key] = _build(sig_scale)
    nc = _cache[key]

    def shard(c):
        xs = x[c * TOK_PC : (c + 1) * TOK_PC]  # [1024, D]
        P = np.rint(xs.T / np.float32(s_x)).astype(np.int16)  # [D, 1024]
        return np.ascontiguousarray(
            P.reshape(KC, 128, NBUF, XBUF_T)
            .transpose(2, 1, 0, 3)
            .reshape(NBUF * 128, KC * XBUF_T)
        )

    with ThreadPoolExecutor(N_CORES) as ex:
        packed = list(ex.map(shard, range(N_CORES)))

    ones_host = np.ones((1, 128), dtype=np.float32)
    in_maps = [
        {
            "xh": packed[c],
            "wh": wh_host,
            "wl": wl_host,
            "bias_p": bias_row,
            "ones_p": ones_host,
        }
        for c in range(N_CORES)
    ]
    res = run_bass_kernel_spmd(nc, in_maps, list(range(N_CORES)))
    LAST_RESULT = res
    w = np.concatenate([res.results[c]["w_out"] for c in range(N_CORES)], axis=0)
    idx = np.concatenate([res.results[c]["idx_out"] for c in range(N_CORES)], axis=0)
    return w, idx.astype(np.int32)


# revision 28
# speedup vs baseline: 1.3576x; 1.0042x over previous
"""MoE gate routing kernel for Trainium2 (8 NeuronCores, data-parallel tokens).

scores = sigmoid(x @ W.T + b); group top-2-sum over 8 groups of 32 picks one
group; top-8 experts inside it; weights = renormalized sigmoid * 2.5.
Returns (w [8192,8] f32, idx [8192,8] i32).

Encoding: the fp32r PE path retains only ~13-14 mantissa bits of its inputs
(any engine write tagged f32r is rounded to that format, and the BIR
verifier requires f32r-tagged producers), so shipping more than ~14 bits of
x is wasted. x therefore ships as a plain int16 payload (2 B/elem) and is
cast on-device to f32r by one tensor_scalar per quarter-buffer (no other
reconstruction work). W ships as int16 + int8 (3 B/elem), reconstructed to
full f32 in a staging tile and rounded once into the f32r W buffer, so its
effective precision equals a raw-f32 DMA at 25% fewer bytes. All scales
fold into the sigmoid scale and a host-precomputed bias row (in payload
units) injected via a ones-column matmul.

Per-core DMA is 2.52 MB = ~57 us at the 360 GB/s DMA ceiling, which is the
bottleneck; PE (fp32r, full 256-wide output rows) needs ~49 us and the
vector engines ~40 us, so both stay off the critical path.
"""

import sys

sys.path.insert(0, "/opt/trn_rl_repo")

from concurrent.futures import ThreadPoolExecutor

import numpy as np

import concourse.bass as bass
from concourse import bacc
import concourse.mybir as mybir
from concourse.bass_utils import run_bass_kernel_spmd
from concourse.tile import TileContext

N_CORES = 8
N_TOK = 8192
TOK_PC = N_TOK // N_CORES  # 1024
D = 7168
E = 256
G = 8
EPG = E // G
TOPK = 8
ROUTE_SCALE = 2.5
KC = D // 128  # 56 k-chunks
KQ = KC // 4  # 14 per quarter
KE = KC // 8  # 7 per eighth (last-buffer tail pieces)
XBUF_T = 128
NBUF = TOK_PC // XBUF_T  # 8

f32 = mybir.dt.float32
f32r = mybir.dt.float32r
i16 = mybir.dt.int16
i8 = mybir.dt.int8
u16 = mybir.dt.uint16
i32 = mybir.dt.int32
u32 = mybir.dt.uint32
AX = mybir.AxisListType
OP = mybir.AluOpType
ACTF = mybir.ActivationFunctionType

_cache = {}
LAST_RESULT = None

# cast engine per (buffer, quarter): phase-A buffers 0-2 pace PE from ACT,
# 3-4 from DVE (behind the w-recon ops there); streaming buffers split
def _cast_eng(s, q):
    if s <= 2:
        return "act"
    if s == NBUF - 1 and q == 3:
        return "dve"
    return "dve" if q in (0, 2) else "act"
CAST_ENG = [[_cast_eng(s, q) for q in range(4)] for s in range(NBUF)]


def _build(sig_scale):
    nc = bacc.Bacc(None, target_bir_lowering=False)

    xh = nc.declare_dram_parameter("xh", [NBUF * 128, KC * XBUF_T], i16, isOutput=False)
    wh = nc.declare_dram_parameter("wh", [D, E], i16, isOutput=False)
    wl = nc.declare_dram_parameter("wl", [D // 2, 2 * E], i8, isOutput=False)
    bias_p = nc.declare_dram_parameter("bias_p", [1, E], f32, isOutput=False)
    ones_p = nc.declare_dram_parameter("ones_p", [1, 128], f32, isOutput=False)
    w_out = nc.declare_dram_parameter("w_out", [TOK_PC, TOPK], f32, isOutput=True)
    idx_out = nc.declare_dram_parameter("idx_out", [TOK_PC, TOPK], i32, isOutput=True)

    xh_v = xh.rearrange("(b p) (c n) -> b p c n", p=128, c=KC)
    wh_v = wh.rearrange("(c p) e -> p c e", p=128)
    wl_v = wl.rearrange("(c2 p) (two e) -> p c2 two e", p=128, two=2)

    with TileContext(nc) as tc:
        with (
            tc.tile_pool(name="const", bufs=1) as cpool,
            tc.tile_pool(name="xh", bufs=9) as hpool,
            tc.tile_pool(name="x32", bufs=5) as xpool,
            tc.tile_pool(name="wst", bufs=3) as wpool,
            tc.tile_pool(name="wstg", bufs=2) as gpool,
            tc.tile_pool(name="sb", bufs=2) as spool,
            tc.tile_pool(name="small", bufs=3) as mpool,
            tc.tile_pool(name="psum", bufs=8, space="PSUM") as ppool,
        ):
            w32 = cpool.tile([128, KC, E], f32r)
            bias_sb = cpool.tile([1, E], f32r)
            ones_sb = cpool.tile([1, 128], f32r)
            w8_all = cpool.tile([128, NBUF, TOPK], f32)
            idx_all = cpool.tile([128, NBUF, TOPK], u32)

            def dma_x_piece(s, lo, hi):
                """[lo, hi) k-chunk slice of buffer s."""
                ht = hpool.tile([128, KQ, XBUF_T], i16, tag="xh")
                nc.sync.dma_start(out=ht[:, : hi - lo, :], in_=xh_v[s, :, lo:hi, :])
                return ht

            KW = 8  # chunks per w piece (aligned to the wl row-pair packing)

            def dma_w_piece(e):
                sl = slice(e * KW, (e + 1) * KW)
                wht = wpool.tile([128, KW, E], i16, tag="wh")
                nc.sync.dma_start(out=wht, in_=wh_v[:, sl, :])
                wlt = wpool.tile([128, KW, E], i8, tag="wl")
                lv = wl_v[:, e * KW // 2 : (e + 1) * KW // 2, :, :]
                nc.sync.dma_start(
                    out=wlt.rearrange("p (a two) e -> p a two e", two=2), in_=lv
                )
                return wht, wlt

            def recon_w_piece(e, pieces):
                """w32[e] = round_f32r(wh*256 + wl): full-f32 staging, one
                rounding at the final tt write so the int8 residual survives.
                tt alternates DVE/Pool so reconstruction keeps up with the
                wire; the first two pieces go in halves to cut PE's startup
                latency."""
                wht, wlt = pieces
                stg = gpool.tile([128, KW, E], f32, tag="wstg")
                for h in range(2):
                    hs = slice(h * KW // 2, (h + 1) * KW // 2)
                    gsl = slice(e * KW + h * KW // 2, e * KW + (h + 1) * KW // 2)
                    nc.vector.tensor_scalar(
                        stg[:, hs, :], wht[:, hs, :], 256.0, None, op0=OP.mult
                    )
                    eng = nc.vector if h == 0 else nc.gpsimd
                    eng.tensor_tensor(
                        out=w32[:, gsl, :], in0=stg[:, hs, :], in1=wlt[:, hs, :],
                        op=OP.add,
                    )

            def cast_piece(ht, s, Q, nchunk):
                x32q = xpool.tile([128, KQ, XBUF_T], f32r, tag="x32")
                src = ht[:, :nchunk, :]
                dst = x32q[:, :nchunk, :]
                if CAST_ENG[s][Q] == "act":
                    nc.scalar.mul(dst, src, 1.0)
                else:
                    nc.vector.tensor_scalar(dst, src, 1.0, None, op0=OP.mult)
                return x32q

            # ---- prologue ----
            nc.scalar.dma_start(out=ones_sb, in_=ones_p[:, :].bitcast(f32r))
            warm = cpool.tile([1, 16], f32)
            nc.vector.memset(warm, 0.0)
            nc.scalar.activation(out=warm, in_=warm, func=ACTF.Sigmoid)
            nc.scalar.dma_start(out=bias_sb, in_=bias_p[:, :].bitcast(f32r))
            # all 8 buffers sweep in one diagonal wavefront; buffer 7's
            # last quarter is two eighths to shorten the tail chain. w pieces
            # are hand-interleaved into the same queue so the wire delivers
            # each one just before PE first needs it.
            pieces = {}  # (s, lo) -> (tile, lo, hi)
            wv = {}
            for s in range(NBUF):
                for Q in range(4):
                    if s == NBUF - 1 and Q == 3:
                        wv.setdefault(s + Q, []).append((s, 42, 49))
                        wv.setdefault(s + Q, []).append((s, 49, 56))
                    elif s == NBUF - 2 and Q == 3:
                        # buffer 6 finishes a wave early so its topk chain
                        # clears DVE before buffer 7's
                        wv.setdefault(s + Q - 1, []).append((s, Q * KQ, (Q + 1) * KQ))
                    else:
                        wv.setdefault(s + Q, []).append((s, Q * KQ, (Q + 1) * KQ))
            xorder = [sq for w_i in sorted(wv) for sq in wv[w_i]]
            w_after = {0: [0, 1], 1: [2, 3], 2: [4], 3: [5], 4: [6]}
            we = {}
            for i, (s, lo, hi) in enumerate(xorder):
                pieces[(s, lo)] = (dma_x_piece(s, lo, hi), lo, hi)
                for e in w_after.get(i, []):
                    we[e] = dma_w_piece(e)

            # PSUM accumulation groups open lazily: each buffer's bias
            # rank-1 matmul is issued right before its first chunk matmul so
            # it is decoded while PE is already busy (full p-state pricing)
            psd = {}
            bias_done = set()

            def open_ps(s):
                ps_s = ppool.tile([128, E], f32, tag="ps")
                psd[s] = ps_s
                nc.tensor.matmul(
                    out=ps_s, lhsT=ones_sb, rhs=bias_sb, start=True, stop=False,
                    skip_group_check=True,
                )

            w_reconned = [False] * 14

            def need_w(cc):
                e = cc // KW
                if not w_reconned[e]:
                    w_reconned[e] = True
                    recon_w_piece(e, we[e])

            def mm_piece(s, lo, hi, ht):
                if s not in bias_done:
                    bias_done.add(s)
                    open_ps(s)
                Q = min(lo // KQ, 3)
                x32q = cast_piece(ht, s, Q, hi - lo)
                for c in range(hi - lo):
                    cc = lo + c
                    need_w(cc)
                    nc.tensor.matmul(
                        out=psd[s],
                        lhsT=x32q[:, c, :],
                        rhs=w32[:, cc, :],
                        start=False,
                        stop=(cc == KC - 1),
                        skip_group_check=True,
                    )

            def topk_buf(s):
                sig = spool.tile([128, G, EPG], f32, tag="sig")
                sig_flat = sig.rearrange("p g e -> p (g e)")
                nc.scalar.activation(
                    out=sig_flat, in_=psd.pop(s), func=ACTF.Sigmoid, scale=sig_scale
                )

                m1 = mpool.tile([128, G], f32, tag="m1")
                nc.vector.tensor_reduce(out=m1, in_=sig, axis=AX.X, op=OP.max)
                scr = spool.tile([128, G, EPG], f32, tag="scr")
                nc.vector.match_replace(
                    out=scr.rearrange("p g e -> p (g e)"),
                    in_to_replace=m1,
                    in_values=sig_flat,
                    imm_value=-1e30,
                )
                gs = mpool.tile([128, G], f32, tag="gs")
                nc.vector.tensor_reduce(out=gs, in_=scr, axis=AX.X, op=OP.max)
                nc.vector.tensor_add(gs, gs, m1)

                gmax = mpool.tile([128, 1], f32, tag="gmax")
                nc.vector.tensor_reduce(out=gmax, in_=gs, axis=AX.X, op=OP.max)
                eq = mpool.tile([128, G, 1], f32, tag="eq")
                nc.vector.tensor_scalar(eq[:, :, 0], gs, gmax, None, op0=OP.is_ge)
                masked = spool.tile([128, G, EPG], f32, tag="masked")
                ba, bb = bass.broadcast_tensor_aps(sig[:, :, :], eq[:, :, :])
                nc.vector.tensor_tensor(out=masked, in0=ba, in1=bb, op=OP.mult)
                masked_flat = masked.rearrange("p g e -> p (g e)")

                vals8 = mpool.tile([128, TOPK], f32, tag="vals8")
                nc.vector.max(out=vals8, in_=masked_flat)
                nc.vector.max_index(
                    out=idx_all[:, s, :], in_max=vals8, in_values=masked_flat
                )
                if s == NBUF - 1:
                    iv = idx_out.rearrange("(s p) k -> p s k", p=128)
                    nc.sync.dma_start(
                        out=iv[:, NBUF - 1 :, :],
                        in_=idx_all[:, NBUF - 1 :, :].bitcast(i32),
                    )
                ssum = mpool.tile([128, 1], f32, tag="ssum")
                nc.vector.tensor_reduce(out=ssum, in_=vals8, axis=AX.X, op=OP.add)
                rcp = mpool.tile([128, 1], f32, tag="rcp")
                nc.vector.reciprocal(out=rcp, in_=ssum)
                nc.vector.tensor_scalar(
                    w8_all[:, s, :], vals8, rcp, ROUTE_SCALE, op0=OP.mult, op1=OP.mult
                )

            # ---- wavefront sweep ----
            for w_i in sorted(wv):
                for s, lo, hi in wv[w_i]:
                    ht, lo2, hi2 = pieces.pop((s, lo))
                    mm_piece(s, lo2, hi2, ht)
                if w_i == 8:
                    topk_buf(5)
                    topk_buf(6)
                elif w_i == 9:
                    pass
                elif 3 <= w_i < 3 + NBUF:
                    topk_buf(w_i - 3)
                if w_i - 3 == NBUF - 2:
                    w_out_v = w_out.rearrange("(s p) k -> p s k", p=128)
                    idx_out_v = idx_out.rearrange("(s p) k -> p s k", p=128)
                    nc.sync.dma_start(
                        out=idx_out_v[:, : NBUF - 1, :],
                        in_=idx_all[:, : NBUF - 1, :].bitcast(i32),
                    )
                    nc.sync.dma_start(
                        out=w_out_v[:, : NBUF - 1, :], in_=w8_all[:, : NBUF - 1, :]
                    )

            w_out_v = w_out.rearrange("(s p) k -> p s k", p=128)
            nc.scalar.dma_start(
                out=w_out_v[:, NBUF - 1 :, :], in_=w8_all[:, NBUF - 1 :, :]
            )
    nc.compile()
    return nc


def kernel(x, weight, bias):
    global LAST_RESULT
    x = np.asarray(x, dtype=np.float32)
    weight = np.asarray(weight, dtype=np.float32)
    bias = np.asarray(bias, dtype=np.float32).reshape(1, E)

    s_x = float(np.abs(x).max()) / 32767.0
    wT = weight.T  # [D, E]
    s_w = float(np.abs(wT).max()) / float((1 << 23) - 256)
    Q = np.rint(wT.astype(np.float64) / s_w).astype(np.int64)
    sig_scale = float(np.float32(s_x * s_w))

    Hw = (Q + 128) >> 8
    Lw = (Q - (Hw << 8)).astype(np.int8)
    wh_host = np.ascontiguousarray(Hw.astype(np.int16))
    # wl rows packed in chunk-pairs so each DMA element is 512B
    Lw3 = Lw.reshape(KC, 128, E)
    wl_host = np.ascontiguousarray(
        np.concatenate([Lw3[0::2], Lw3[1::2]], axis=2).reshape(D // 2, 2 * E)
    )
    bias_row = (bias.astype(np.float64) / (s_x * s_w)).astype(np.float32)

    key = sig_scale
    if key not in _cache:
        _cache.clear()
        _cache[key] = _build(sig_scale)
    nc = _cache[key]

    def shard(c):
        xs = x[c * TOK_PC : (c + 1) * TOK_PC]  # [1024, D]
        P = np.rint(xs.T / np.float32(s_x)).astype(np.int16)  # [D, 1024]
        return np.ascontiguousarray(
            P.reshape(KC, 128, NBUF, XBUF_T)
            .transpose(2, 1, 0, 3)
            .reshape(NBUF * 128, KC * XBUF_T)
        )

    with ThreadPoolExecutor(N_CORES) as ex:
        packed = list(ex.map(shard, range(N_CORES)))

    ones_host = np.ones((1, 128), dtype=np.float32)
    in_maps = [
        {
            "xh": packed[c],
            "wh": wh_host,
            "wl": wl_host,
            "bias_p": bias_row,
            "ones_p": ones_host,
        }
        for c in range(N_CORES)
    ]
    res = run_bass_kernel_spmd(nc, in_maps, list(range(N_CORES)))
    LAST_RESULT = res
    w = np.concatenate([res.results[c]["w_out"] for c in range(N_CORES)], axis=0)
    idx = np.concatenate([res.results[c]["idx_out"] for c in range(N_CORES)], axis=0)
    return w, idx.astype(np.int32)
